# revision 21
# baseline (speedup 1.0000x reference)
"""MHA kernel for trn2: 8-core SPMD, core c = (batch c//2, head-group c%2 of 8 heads).

Per-core pipeline (all shapes hardcoded for B=4, S=2048, HIDDEN=1024, H=16, DK=DV=64):
  Prologue: K^T per head-pair [128, S] (bf16, bias via ACT Identity per-partition
            bias on PSUM evac), ones-augmented V [sk, 8, 65] (no bias: bv folds
            into bo on the host), Q^T for sq-block 0 only.
  Main loop: per sq-block j (512), per (head hl, sk-tile-pair tt):
            scores^T = K Q^T (2 matmuls), exp on ACT (scale=1/8), mask multiply
            on DVE (bf16, all-SBUF 2x mode), PV accumulation lagging 2 tiles to
            hide the exp->mask latency; the denominator rides the 65th V column.
            Drip-fed per iteration: Q^T projection for block j+1, the previous
            block's normalization (one-hot-selector broadcast matmul + DVE mul)
            and out-projection (pair-packed lhsT, K=128), mask prefetch for j+1.
  Host sums the 2 group partials per batch + bo + bv@wo.
"""

import numpy as np
import ml_dtypes

import concourse.bacc as bacc
import concourse.mybir as mybir
import concourse.tile as tile
from concourse.bass_utils import run_bass_kernel_spmd

B, S, HID, H = 4, 2048, 1024, 16
DK = DV = 64
G = 2              # head groups per batch (8 heads each)
HPC, PAIRS = 8, 4  # heads / head-pairs per core
SQB = 512          # sq block
NJ = S // SQB      # 4
NT = S // 128      # 16 sk tiles
KTN = HID // 128   # 8 hidden k-tiles

F32 = mybir.dt.float32
F32R = mybir.dt.float32r
BF16 = mybir.dt.bfloat16
AF = mybir.ActivationFunctionType


_NC = None


def _build_nc():
    nc = bacc.Bacc("TRN2")
    xq_d = nc.declare_dram_parameter("xqT", [NJ, 128, KTN, SQB], BF16,
                                     isOutput=False)
    xk_d = nc.declare_dram_parameter("xkT", [NJ, 128, KTN, SQB], BF16,
                                     isOutput=False)
    xv_d = nc.declare_dram_parameter("xvT", [NJ, 128, KTN, SQB], BF16,
                                     isOutput=False)
    mk_d = nc.declare_dram_parameter("maskJ", [NJ, 128, NT, SQB], BF16,
                                     isOutput=False)
    wq_d = nc.declare_dram_parameter("wq", [128, KTN, 512], BF16, isOutput=False)
    wk_d = nc.declare_dram_parameter("wk", [128, KTN, 512], BF16, isOutput=False)
    wv_d = nc.declare_dram_parameter("wv", [128, KTN, 512], BF16, isOutput=False)
    bq_d = nc.declare_dram_parameter("bqT", [128, PAIRS], F32, isOutput=False)
    bk_d = nc.declare_dram_parameter("bkT", [128, PAIRS], F32, isOutput=False)
    wo_d = nc.declare_dram_parameter("wo", [PAIRS, 128, HID], BF16, isOutput=False)
    sel_d = nc.declare_dram_parameter("sel", [HPC, HPC * DV], F32R, isOutput=False)
    out_d = nc.declare_dram_parameter("out", [S, HID], F32, isOutput=True)

    with tile.TileContext(nc) as tc:
        with tc.tile_pool(name="persist", bufs=1) as PP, \
             tc.tile_pool(name="wqp", bufs=1) as WQP, \
             tc.tile_pool(name="mskp", bufs=2) as MP, \
             tc.tile_pool(name="xpool", bufs=2) as XP, \
             tc.tile_pool(name="wpool", bufs=2) as WP, \
             tc.tile_pool(name="ptp", bufs=3) as PTP, \
             tc.tile_pool(name="oup", bufs=2) as OUP, \
             tc.tile_pool(name="onp", bufs=2) as ONP, \
             tc.tile_pool(name="dnp", bufs=2) as DNP, \
             tc.tile_pool(name="dtp", bufs=2) as DTP, \
             tc.tile_pool(name="rcp", bufs=2) as RCP, \
             tc.tile_pool(name="obp", bufs=2) as OBP:
            qT = PP.tile([128, PAIRS, S], BF16, name="qT")
            kT = PP.tile([128, PAIRS, S], BF16, name="kT")
            vA = PP.tile([128, NT, HPC, DV + 1], BF16, name="vA")
            bq_sb = PP.tile([128, PAIRS], F32, name="bq_sb")
            bk_sb = PP.tile([128, PAIRS], F32, name="bk_sb")
            nc.vector.memset(vA[:, :, :, DV:DV + 1], 1.0)
            # one-hot selector rows: sel[k, hl*64:(hl+1)*64] = (k == hl)
            sel = PP.tile([HPC, HPC * DV], F32R, name="sel")
            nc.sync.dma_start(sel[:], sel_d[:])
            nc.sync.dma_start(bq_sb[:], bq_d[:])
            nc.sync.dma_start(bk_sb[:], bk_d[:])
            # bulky prefetches go on the idle gpsimd queue so the first
            # K-projection matmul isn't stuck behind them
            wo_sb = PP.tile([128, PAIRS, HID], BF16, name="wo_sb")
            for hp in range(PAIRS):
                nc.gpsimd.dma_start(wo_sb[:, hp, :], wo_d[hp])
            msk0 = MP.tile([128, NT, SQB], BF16, name="msk")
            nc.gpsimd.dma_start(msk0[:], mk_d[0])
            wq_sb = WQP.tile([128, KTN, 512], BF16, name="wq_sb")
            nc.gpsimd.dma_start(wq_sb[:], wq_d[:])

            # ---------------- Prologue: K, V, Q(j=0) projections ----------
            with tc.tile_pool(name="prps", bufs=2, space="PSUM") as PR:
                def qk_proj(xd, w_sb, brow, dstT, n, dma):
                    x_sb = XP.tile([128, KTN, SQB], BF16, name="x_sb")
                    dma(x_sb[:], xd[n])
                    for hp in range(PAIRS):
                        ps = PR.tile([128, SQB], F32, name="ps_qk")
                        for k in range(KTN):
                            nc.tensor.matmul(
                                ps[:], w_sb[:, k, hp * 128:(hp + 1) * 128],
                                x_sb[:, k, :], start=(k == 0), stop=(k == KTN - 1))
                        nc.scalar.activation(
                            dstT[:, hp, n * SQB:(n + 1) * SQB], ps[:],
                            AF.Identity, bias=brow[:, hp:hp + 1])

                wk_sb = WP.tile([128, KTN, 512], BF16, name="w_sb")
                nc.sync.dma_start(wk_sb[:], wk_d[:])
                for n in range(NJ):
                    qk_proj(xk_d, wk_sb, bk_sb, kT, n,
                            nc.sync.dma_start if n % 2 == 0
                            else nc.scalar.dma_start)

                # V: out[sk 128, head, dv] += x.T @ wv (no bias)
                wv_sb = WP.tile([128, KTN, 512], BF16, name="w_sb")
                nc.scalar.dma_start(wv_sb[:], wv_d[:])
                for c in range(4):
                    x_sb = XP.tile([128, KTN, SQB], BF16, name="x_sb")
                    (nc.sync.dma_start if c % 2 == 0
                     else nc.scalar.dma_start)(x_sb[:], xv_d[c])
                    for stl in range(4):
                        st = c * 4 + stl
                        ps = PR.tile([128, HPC, DV], F32, name="ps_v")
                        for k in range(KTN):
                            nc.tensor.matmul(
                                ps[:], x_sb[:, k, stl * 128:(stl + 1) * 128],
                                wv_sb[:, k, :], start=(k == 0),
                                stop=(k == KTN - 1))
                        nc.vector.tensor_copy(vA[:, st, :, 0:DV], ps[:])

                for n in range(NJ):
                    qk_proj(xq_d, wq_sb, bq_sb, qT, n,
                            nc.sync.dma_start if n % 2 == 0
                            else nc.scalar.dma_start)

            # ---------------- Main loop: attention + drip-fed extras --------
            with tc.tile_pool(name="scps", bufs=2, space="PSUM") as SCP, \
                 tc.tile_pool(name="pvps", bufs=2, space="PSUM") as PVP, \
                 tc.tile_pool(name="opps", bufs=2, space="PSUM") as OPP:

                def tail_steps(j, rc8, oU, oN):
                    steps = []

                    def ptile():
                        return OPP.tile([128, SQB], F32, name="op")

                    for hl in range(HPC):
                        hp, r = divmod(hl, 2)
                        pb = 64 * r

                        def s_bc(hl=hl, hp=hp, pb=pb):
                            bc = ptile()
                            nc.tensor.matmul(bc[0:DV, :],
                                             sel[:, hl * DV:(hl + 1) * DV],
                                             rc8[:], start=True, stop=True)
                            nc.vector.tensor_mul(oN[pb:pb + DV, hp, :],
                                                 oU[pb:pb + DV, hp, :],
                                                 bc[0:DV, :])
                        steps.append(s_bc)
                    for stl in range(4):
                        for nn in range(2):
                            def s_op(stl=stl, nn=nn):
                                st = 4 * j + stl
                                op = ptile()
                                for hp in range(PAIRS):
                                    nc.tensor.matmul(
                                        op[:],
                                        oN[:, hp, stl * 128:(stl + 1) * 128],
                                        wo_sb[:, hp, nn * SQB:(nn + 1) * SQB],
                                        start=(hp == 0), stop=(hp == PAIRS - 1))
                                ob = OBP.tile([128, SQB], F32, name="ob")
                                nc.vector.tensor_copy(ob[:], op[:])
                                nc.sync.dma_start(
                                    out_d[st * 128:(st + 1) * 128,
                                          nn * SQB:(nn + 1) * SQB], ob[:])
                            steps.append(s_op)
                    return steps

                def emit_recip(denj):
                    rc8 = RCP.tile([HPC, SQB], F32R, name="rc8")
                    with nc.allow_low_precision(reason="f32r recip"):
                        nc.vector.reciprocal(rc8[:], denj[:])
                    return rc8

                def emit_evac(ev):
                    cpv, chl, cpb, chp, cdenj, coU = ev
                    dtmp = DTP.tile([1, SQB], F32, name="dtmp")
                    nc.vector.tensor_copy(dtmp[:], cpv[DV:DV + 1, :])
                    nc.sync.dma_start(cdenj[chl:chl + 1, :], dtmp[:])
                    nc.vector.tensor_copy(coU[cpb:cpb + DV, chp, :],
                                          cpv[0:DV, :])

                class Carry:
                    __slots__ = ("pv", "pt", "tt", "hl", "pb", "hp",
                                 "denj", "oU", "fin")

                    def __init__(self, pt, tt, hl, pb, hp, denj, oU, fin):
                        self.pv = None
                        self.pt, self.tt, self.hl = pt, tt, hl
                        self.pb, self.hp = pb, hp
                        self.denj, self.oU, self.fin = denj, oU, fin

                def emit_pv(c, pvref):
                    # pvref[hl] caches the PVP tile for this head
                    if c.tt == 0:
                        pvref[c.hl] = PVP.tile([DV + 1, SQB], F32, name="pv")
                    pv = pvref[c.hl]
                    for u in range(2):
                        nc.tensor.matmul(
                            pv[:], vA[:, 2 * c.tt + u, c.hl, :], c.pt[:, u, :],
                            start=(c.tt == 0 and u == 0),
                            stop=(c.fin and u == 1))
                    if c.fin:
                        return (pv, c.hl, c.pb, c.hp, c.denj, c.oU)
                    return None

                carries = []   # FIFO of pending pv emissions (depth 3)
                pvref = {}
                steps = []
                pend = None

                def drain_one():
                    if len(carries) >= 3:
                        ev = emit_pv(carries.pop(0), pvref)
                        if ev is not None:
                            emit_evac(ev)

                for j in range(NJ):
                    if j == 0:
                        msk = msk0
                    else:
                        msk = msk_next
                    denj = DNP.tile([HPC, SQB], F32, name="denj")
                    oU = OUP.tile([128, PAIRS, SQB], BF16, name="oU")
                    oN = ONP.tile([128, PAIRS, SQB], BF16, name="oN")
                    for hl in range(HPC):
                        hp, r = divmod(hl, 2)
                        pb = 64 * r
                        for tt in range(NT // 2):
                            gi = hl * (NT // 2) + tt
                            sc = SCP.tile([128, 2, SQB], F32, name="sc")
                            for u in range(2):
                                t = 2 * tt + u
                                nc.tensor.matmul(
                                    sc[:, u, :],
                                    kT[pb:pb + DK, hp, t * 128:(t + 1) * 128],
                                    qT[pb:pb + DK, hp, j * SQB:(j + 1) * SQB],
                                    start=True, stop=True)
                            drain_one()
                            if tt % 2 == 0:
                                pt2 = PTP.tile([128, 4, SQB], BF16, name="pt")
                            uo = 2 * (tt % 2)
                            nc.scalar.activation(pt2[:, uo:uo + 2, :], sc[:],
                                                 AF.Exp, scale=0.125)
                            if tt % 2 == 1:
                                # one batched mult per tile-pair amortizes the
                                # DVE per-instruction overhead
                                nc.vector.tensor_mul(
                                    pt2[:], pt2[:],
                                    msk[:, 2 * tt - 2:2 * tt + 2, :])
                            carries.append(
                                Carry(pt2[:, uo:uo + 2, :], tt, hl, pb, hp,
                                      denj, oU, tt == NT // 2 - 1))
                            # drip-fed extras, one per iteration slot
                            if gi == 3 and pend is not None:
                                rc8 = emit_recip(pend[1])
                                steps = tail_steps(pend[0], rc8,
                                                   pend[2], pend[3])
                            elif gi == 41 and j < NJ - 1:
                                msk_next = MP.tile([128, NT, SQB], BF16,
                                                   name="msk")
                                nc.gpsimd.dma_start(msk_next[:],
                                                    mk_d[j + 1])
                            elif gi >= 6 and gi % 4 == 2 and steps:
                                steps.pop(0)()
                    while steps:
                        steps.pop(0)()
                    pend = (j, denj, oU, oN)
                # final block's tail, nothing left to overlap with
                while carries:
                    ev = emit_pv(carries.pop(0), pvref)
                    if ev is not None:
                        emit_evac(ev)
                rc8 = emit_recip(pend[1])
                for s in tail_steps(pend[0], rc8, pend[2], pend[3]):
                    s()
    nc.finalize()
    return nc


def get_nc():
    global _NC
    if _NC is None:
        _NC = _build_nc()
    return _NC


def make_in_maps(q_hidden_inputs, k_hidden_inputs, v_hidden_inputs, mask,
                 wq, bq, wk, bk, wv, bv, wo, bo):
    f32 = np.float32
    bf16 = ml_dtypes.bfloat16
    in_maps = []
    per_batch = []
    sel = np.zeros((HPC, HPC * DV), dtype=f32)
    for hl in range(HPC):
        sel[hl, hl * DV:(hl + 1) * DV] = 1.0
    def x_tile(x):
        # [NJ, 128, KTN, SQB] with x3[n, p, k, s] = x[n*SQB+s, k*128+p]
        return np.ascontiguousarray(
            np.asarray(x).reshape(NJ, SQB, KTN, 128).transpose(0, 3, 2, 1)
        ).astype(bf16)

    def w_tile(w_grp):
        # [128, KTN, 512] with w2[p, k, n] = w_grp[k*128+p, n]
        return np.ascontiguousarray(
            w_grp.reshape(KTN, 128, 512).transpose(1, 0, 2)).astype(bf16)

    for b in range(B):
        xqT = x_tile(q_hidden_inputs[b])
        xkT = x_tile(k_hidden_inputs[b])
        xvT = x_tile(v_hidden_inputs[b])
        maskT = mask[b].T.astype(bf16)                        # [sk, sq]
        # maskJ[j, p, t, s] = maskT[t*128+p, j*512+s]
        maskJ = np.ascontiguousarray(
            maskT.reshape(NT, 128, NJ, SQB).transpose(2, 1, 0, 3))
        per_batch.append((xqT, xkT, xvT, maskJ))
    for c in range(2 * B):
        b, g = divmod(c, 2)
        xqT, xkT, xvT, maskJ = per_batch[b]
        hs = slice(g * HPC, (g + 1) * HPC)
        in_maps.append({
            "xqT": xqT, "xkT": xkT, "xvT": xvT, "maskJ": maskJ,
            "wq": w_tile(wq[hs].transpose(1, 0, 2).reshape(HID, 512)),
            "wk": w_tile(wk[hs].transpose(1, 0, 2).reshape(HID, 512)),
            "wv": w_tile(wv[hs].transpose(1, 0, 2).reshape(HID, 512)),
            # bqT[p, hp] = bq[g*8 + 2*hp + p//64, p%64]
            "bqT": np.ascontiguousarray(
                bq[hs].reshape(PAIRS, 128).T, dtype=f32),
            "bkT": np.ascontiguousarray(
                bk[hs].reshape(PAIRS, 128).T, dtype=f32),
            "wo": np.ascontiguousarray(
                wo[g * 512:(g + 1) * 512, :].reshape(PAIRS, 128, HID)
            ).astype(bf16),
            "sel": sel,
        })
    return in_maps


def assemble(results, bv, wo, bo):
    # v-bias contribution folds through softmax: out_h = rawPV_h/denom + bv_h,
    # so sum_h bv_h @ wo_h is a constant row added once per batch.
    bvw = (bv.astype(np.float32).reshape(-1) @ wo.astype(np.float32)
           + bo.astype(np.float32))
    out = np.empty((B, S, HID), dtype=np.float32)
    for b in range(B):
        out[b] = results[2 * b]["out"] + results[2 * b + 1]["out"] \
            + bvw[None, :]
    return out


def run(inputs, trace=False, **kw):
    nc = get_nc()
    in_maps = make_in_maps(**inputs)
    bkr = run_bass_kernel_spmd(nc, in_maps, list(range(2 * B)), trace=trace, **kw)
    return assemble(bkr.results, np.asarray(inputs["bv"]),
                    np.asarray(inputs["wo"]), np.asarray(inputs["bo"])), bkr


def kernel(**inputs):
    out, _ = run(inputs, trace=False)
    return out


# revision 22
# speedup vs baseline: 1.0074x; 1.0074x over previous
"""MHA kernel for trn2: 8-core SPMD, core c = (batch c//2, head-group c%2 of 8 heads).

Per-core pipeline (all shapes hardcoded for B=4, S=2048, HIDDEN=1024, H=16, DK=DV=64):
  Prologue: K^T per head-pair [128, S] (bf16, bias via ACT Identity per-partition
            bias on PSUM evac), ones-augmented V [sk, 8, 65] (no bias: bv folds
            into bo on the host), Q^T for sq-block 0 only.
  Main loop: per sq-block j (512), per (head hl, sk-tile-pair tt):
            scores^T = K Q^T (2 matmuls), exp on ACT (scale=1/8), mask multiply
            on DVE (bf16, all-SBUF 2x mode), PV accumulation lagging 2 tiles to
            hide the exp->mask latency; the denominator rides the 65th V column.
            Drip-fed per iteration: Q^T projection for block j+1, the previous
            block's normalization (one-hot-selector broadcast matmul + DVE mul)
            and out-projection (pair-packed lhsT, K=128), mask prefetch for j+1.
  Host sums the 2 group partials per batch + bo + bv@wo.
"""

import numpy as np
import ml_dtypes

import concourse.bacc as bacc
import concourse.mybir as mybir
import concourse.tile as tile
from concourse.bass_utils import run_bass_kernel_spmd

B, S, HID, H = 4, 2048, 1024, 16
DK = DV = 64
G = 2              # head groups per batch (8 heads each)
HPC, PAIRS = 8, 4  # heads / head-pairs per core
SQB = 512          # sq block
NJ = S // SQB      # 4
NT = S // 128      # 16 sk tiles
KTN = HID // 128   # 8 hidden k-tiles

F32 = mybir.dt.float32
F32R = mybir.dt.float32r
BF16 = mybir.dt.bfloat16
AF = mybir.ActivationFunctionType


_NC = None


def _build_nc():
    nc = bacc.Bacc("TRN2")
    xq_d = nc.declare_dram_parameter("xqT", [NJ, 128, KTN, SQB], BF16,
                                     isOutput=False)
    xk_d = nc.declare_dram_parameter("xkT", [NJ, 128, KTN, SQB], BF16,
                                     isOutput=False)
    xv_d = nc.declare_dram_parameter("xvT", [NJ, 128, KTN, SQB], BF16,
                                     isOutput=False)
    mk_d = nc.declare_dram_parameter("maskJ", [NJ, 128, NT, SQB], BF16,
                                     isOutput=False)
    wq_d = nc.declare_dram_parameter("wq", [128, KTN, 512], BF16, isOutput=False)
    wk_d = nc.declare_dram_parameter("wk", [128, KTN, 512], BF16, isOutput=False)
    wv_d = nc.declare_dram_parameter("wv", [128, KTN, 512], BF16, isOutput=False)
    bq_d = nc.declare_dram_parameter("bqT", [128, PAIRS], F32, isOutput=False)
    bk_d = nc.declare_dram_parameter("bkT", [128, PAIRS], F32, isOutput=False)
    wo_d = nc.declare_dram_parameter("wo", [PAIRS, 128, HID], BF16, isOutput=False)
    sel_d = nc.declare_dram_parameter("sel", [HPC, HPC * DV], F32R, isOutput=False)
    out_d = nc.declare_dram_parameter("out", [S, HID], F32, isOutput=True)

    with tile.TileContext(nc) as tc:
        with tc.tile_pool(name="persist", bufs=1) as PP, \
             tc.tile_pool(name="wqp", bufs=1) as WQP, \
             tc.tile_pool(name="mskp", bufs=2) as MP, \
             tc.tile_pool(name="xpool", bufs=2) as XP, \
             tc.tile_pool(name="wpool", bufs=2) as WP, \
             tc.tile_pool(name="ptp", bufs=3) as PTP, \
             tc.tile_pool(name="oup", bufs=2) as OUP, \
             tc.tile_pool(name="onp", bufs=2) as ONP, \
             tc.tile_pool(name="dnp", bufs=2) as DNP, \
             tc.tile_pool(name="dtp", bufs=2) as DTP, \
             tc.tile_pool(name="rcp", bufs=2) as RCP, \
             tc.tile_pool(name="obp", bufs=2) as OBP:
            qT = PP.tile([128, PAIRS, S], BF16, name="qT")
            kT = PP.tile([128, PAIRS, S], BF16, name="kT")
            vA = PP.tile([128, NT, HPC, DV + 1], BF16, name="vA")
            bq_sb = PP.tile([128, PAIRS], F32, name="bq_sb")
            bk_sb = PP.tile([128, PAIRS], F32, name="bk_sb")
            nc.vector.memset(vA[:, :, :, DV:DV + 1], 1.0)
            sel = PP.tile([HPC, HPC * DV], F32R, name="sel")
            nc.sync.dma_start(bq_sb[:], bq_d[:])
            nc.sync.dma_start(bk_sb[:], bk_d[:])
            wo_sb = PP.tile([128, PAIRS, HID], BF16, name="wo_sb")
            msk0 = MP.tile([128, NT, SQB], BF16, name="msk")
            wq_sb = WQP.tile([128, KTN, 512], BF16, name="wq_sb")

            # ---------------- Prologue: K, V, Q(j=0) projections ----------
            with tc.tile_pool(name="prps", bufs=2, space="PSUM") as PR:
                def qk_proj(xd, w_sb, brow, dstT, n, dma):
                    x_sb = XP.tile([128, KTN, SQB], BF16, name="x_sb")
                    dma(x_sb[:], xd[n])
                    for hp in range(PAIRS):
                        ps = PR.tile([128, SQB], F32, name="ps_qk")
                        for k in range(KTN):
                            nc.tensor.matmul(
                                ps[:], w_sb[:, k, hp * 128:(hp + 1) * 128],
                                x_sb[:, k, :], start=(k == 0), stop=(k == KTN - 1))
                        nc.scalar.activation(
                            dstT[:, hp, n * SQB:(n + 1) * SQB], ps[:],
                            AF.Identity, bias=brow[:, hp:hp + 1])

                wk_sb = WP.tile([128, KTN, 512], BF16, name="w_sb")
                nc.sync.dma_start(wk_sb[:], wk_d[:])
                for n in range(NJ):
                    qk_proj(xk_d, wk_sb, bk_sb, kT, n, nc.sync.dma_start)

                # deferred bulk loads, behind the K-projection x tiles in
                # queue order: needed only from the main loop onwards
                nc.scalar.dma_start(wq_sb[:], wq_d[:])
                nc.scalar.dma_start(msk0[:], mk_d[0])
                for hp in range(PAIRS):
                    nc.scalar.dma_start(wo_sb[:, hp, :], wo_d[hp])
                nc.scalar.dma_start(sel[:], sel_d[:])

                # V: out[sk 128, head, dv] += x.T @ wv (no bias)
                wv_sb = WP.tile([128, KTN, 512], BF16, name="w_sb")
                nc.sync.dma_start(wv_sb[:], wv_d[:])
                for c in range(4):
                    x_sb = XP.tile([128, KTN, SQB], BF16, name="x_sb")
                    nc.sync.dma_start(x_sb[:], xv_d[c])
                    for stl in range(4):
                        st = c * 4 + stl
                        ps = PR.tile([128, HPC, DV], F32, name="ps_v")
                        for k in range(KTN):
                            nc.tensor.matmul(
                                ps[:], x_sb[:, k, stl * 128:(stl + 1) * 128],
                                wv_sb[:, k, :], start=(k == 0),
                                stop=(k == KTN - 1))
                        nc.vector.tensor_copy(vA[:, st, :, 0:DV], ps[:])

                for n in range(NJ):
                    qk_proj(xq_d, wq_sb, bq_sb, qT, n, nc.sync.dma_start)

            # ---------------- Main loop: attention + drip-fed extras --------
            with tc.tile_pool(name="scps", bufs=2, space="PSUM") as SCP, \
                 tc.tile_pool(name="pvps", bufs=2, space="PSUM") as PVP, \
                 tc.tile_pool(name="opps", bufs=2, space="PSUM") as OPP:

                def tail_steps(j, rc8, oU, oN):
                    steps = []

                    def ptile():
                        return OPP.tile([128, SQB], F32, name="op")

                    for hl in range(HPC):
                        hp, r = divmod(hl, 2)
                        pb = 64 * r

                        def s_bc(hl=hl, hp=hp, pb=pb):
                            bc = ptile()
                            nc.tensor.matmul(bc[0:DV, :],
                                             sel[:, hl * DV:(hl + 1) * DV],
                                             rc8[:], start=True, stop=True)
                            nc.vector.tensor_mul(oN[pb:pb + DV, hp, :],
                                                 oU[pb:pb + DV, hp, :],
                                                 bc[0:DV, :])
                        steps.append(s_bc)
                    for stl in range(4):
                        for nn in range(2):
                            def s_op(stl=stl, nn=nn):
                                st = 4 * j + stl
                                op = ptile()
                                for hp in range(PAIRS):
                                    nc.tensor.matmul(
                                        op[:],
                                        oN[:, hp, stl * 128:(stl + 1) * 128],
                                        wo_sb[:, hp, nn * SQB:(nn + 1) * SQB],
                                        start=(hp == 0), stop=(hp == PAIRS - 1))
                                ob = OBP.tile([128, SQB], F32, name="ob")
                                nc.vector.tensor_copy(ob[:], op[:])
                                nc.sync.dma_start(
                                    out_d[st * 128:(st + 1) * 128,
                                          nn * SQB:(nn + 1) * SQB], ob[:])
                            steps.append(s_op)
                    return steps

                def emit_recip(denj):
                    rc8 = RCP.tile([HPC, SQB], F32R, name="rc8")
                    with nc.allow_low_precision(reason="f32r recip"):
                        nc.vector.reciprocal(rc8[:], denj[:])
                    return rc8

                def emit_evac(ev):
                    cpv, chl, cpb, chp, cdenj, coU = ev
                    dtmp = DTP.tile([1, SQB], F32, name="dtmp")
                    nc.vector.tensor_copy(dtmp[:], cpv[DV:DV + 1, :])
                    nc.sync.dma_start(cdenj[chl:chl + 1, :], dtmp[:])
                    nc.vector.tensor_copy(coU[cpb:cpb + DV, chp, :],
                                          cpv[0:DV, :])

                class Carry:
                    __slots__ = ("pv", "pt", "tt", "hl", "pb", "hp",
                                 "denj", "oU", "fin")

                    def __init__(self, pt, tt, hl, pb, hp, denj, oU, fin):
                        self.pv = None
                        self.pt, self.tt, self.hl = pt, tt, hl
                        self.pb, self.hp = pb, hp
                        self.denj, self.oU, self.fin = denj, oU, fin

                def emit_pv(c, pvref):
                    # pvref[hl] caches the PVP tile for this head
                    if c.tt == 0:
                        pvref[c.hl] = PVP.tile([DV + 1, SQB], F32, name="pv")
                    pv = pvref[c.hl]
                    for u in range(2):
                        nc.tensor.matmul(
                            pv[:], vA[:, 2 * c.tt + u, c.hl, :], c.pt[:, u, :],
                            start=(c.tt == 0 and u == 0),
                            stop=(c.fin and u == 1))
                    if c.fin:
                        return (pv, c.hl, c.pb, c.hp, c.denj, c.oU)
                    return None

                carries = []   # FIFO of pending pv emissions (depth 3)
                pvref = {}
                steps = []
                pend = None

                def drain_one():
                    if len(carries) >= 3:
                        ev = emit_pv(carries.pop(0), pvref)
                        if ev is not None:
                            emit_evac(ev)

                for j in range(NJ):
                    if j == 0:
                        msk = msk0
                    else:
                        msk = msk_next
                    denj = DNP.tile([HPC, SQB], F32, name="denj")
                    oU = OUP.tile([128, PAIRS, SQB], BF16, name="oU")
                    oN = ONP.tile([128, PAIRS, SQB], BF16, name="oN")
                    for hl in range(HPC):
                        hp, r = divmod(hl, 2)
                        pb = 64 * r
                        for tt in range(NT // 2):
                            gi = hl * (NT // 2) + tt
                            sc = SCP.tile([128, 2, SQB], F32, name="sc")
                            for u in range(2):
                                t = 2 * tt + u
                                nc.tensor.matmul(
                                    sc[:, u, :],
                                    kT[pb:pb + DK, hp, t * 128:(t + 1) * 128],
                                    qT[pb:pb + DK, hp, j * SQB:(j + 1) * SQB],
                                    start=True, stop=True)
                            drain_one()
                            if tt % 2 == 0:
                                pt2 = PTP.tile([128, 4, SQB], BF16, name="pt")
                            uo = 2 * (tt % 2)
                            nc.scalar.activation(pt2[:, uo:uo + 2, :], sc[:],
                                                 AF.Exp, scale=0.125)
                            if tt % 2 == 1:
                                # one batched mult per tile-pair amortizes the
                                # DVE per-instruction overhead
                                nc.vector.tensor_mul(
                                    pt2[:], pt2[:],
                                    msk[:, 2 * tt - 2:2 * tt + 2, :])
                            carries.append(
                                Carry(pt2[:, uo:uo + 2, :], tt, hl, pb, hp,
                                      denj, oU, tt == NT // 2 - 1))
                            # drip-fed extras, one per iteration slot
                            if gi == 3 and pend is not None:
                                rc8 = emit_recip(pend[1])
                                steps = tail_steps(pend[0], rc8,
                                                   pend[2], pend[3])
                            elif gi == 41 and j < NJ - 1:
                                msk_next = MP.tile([128, NT, SQB], BF16,
                                                   name="msk")
                                nc.gpsimd.dma_start(msk_next[:],
                                                    mk_d[j + 1])
                            elif gi >= 6 and gi % 4 == 2 and steps:
                                steps.pop(0)()
                    while steps:
                        steps.pop(0)()
                    pend = (j, denj, oU, oN)
                # final block's tail, nothing left to overlap with
                while carries:
                    ev = emit_pv(carries.pop(0), pvref)
                    if ev is not None:
                        emit_evac(ev)
                rc8 = emit_recip(pend[1])
                for s in tail_steps(pend[0], rc8, pend[2], pend[3]):
                    s()
    nc.finalize()
    return nc


def get_nc():
    global _NC
    if _NC is None:
        _NC = _build_nc()
    return _NC


def make_in_maps(q_hidden_inputs, k_hidden_inputs, v_hidden_inputs, mask,
                 wq, bq, wk, bk, wv, bv, wo, bo):
    f32 = np.float32
    bf16 = ml_dtypes.bfloat16
    in_maps = []
    per_batch = []
    sel = np.zeros((HPC, HPC * DV), dtype=f32)
    for hl in range(HPC):
        sel[hl, hl * DV:(hl + 1) * DV] = 1.0
    def x_tile(x):
        # [NJ, 128, KTN, SQB] with x3[n, p, k, s] = x[n*SQB+s, k*128+p]
        return np.ascontiguousarray(
            np.asarray(x).reshape(NJ, SQB, KTN, 128).transpose(0, 3, 2, 1)
        ).astype(bf16)

    def w_tile(w_grp):
        # [128, KTN, 512] with w2[p, k, n] = w_grp[k*128+p, n]
        return np.ascontiguousarray(
            w_grp.reshape(KTN, 128, 512).transpose(1, 0, 2)).astype(bf16)

    for b in range(B):
        xqT = x_tile(q_hidden_inputs[b])
        xkT = x_tile(k_hidden_inputs[b])
        xvT = x_tile(v_hidden_inputs[b])
        maskT = mask[b].T.astype(bf16)                        # [sk, sq]
        # maskJ[j, p, t, s] = maskT[t*128+p, j*512+s]
        maskJ = np.ascontiguousarray(
            maskT.reshape(NT, 128, NJ, SQB).transpose(2, 1, 0, 3))
        per_batch.append((xqT, xkT, xvT, maskJ))
    for c in range(2 * B):
        b, g = divmod(c, 2)
        xqT, xkT, xvT, maskJ = per_batch[b]
        hs = slice(g * HPC, (g + 1) * HPC)
        in_maps.append({
            "xqT": xqT, "xkT": xkT, "xvT": xvT, "maskJ": maskJ,
            "wq": w_tile(wq[hs].transpose(1, 0, 2).reshape(HID, 512)),
            "wk": w_tile(wk[hs].transpose(1, 0, 2).reshape(HID, 512)),
            "wv": w_tile(wv[hs].transpose(1, 0, 2).reshape(HID, 512)),
            # bqT[p, hp] = bq[g*8 + 2*hp + p//64, p%64]
            "bqT": np.ascontiguousarray(
                bq[hs].reshape(PAIRS, 128).T, dtype=f32),
            "bkT": np.ascontiguousarray(
                bk[hs].reshape(PAIRS, 128).T, dtype=f32),
            "wo": np.ascontiguousarray(
                wo[g * 512:(g + 1) * 512, :].reshape(PAIRS, 128, HID)
            ).astype(bf16),
            "sel": sel,
        })
    return in_maps


def assemble(results, bv, wo, bo):
    # v-bias contribution folds through softmax: out_h = rawPV_h/denom + bv_h,
    # so sum_h bv_h @ wo_h is a constant row added once per batch.
    bvw = (bv.astype(np.float32).reshape(-1) @ wo.astype(np.float32)
           + bo.astype(np.float32))
    out = np.empty((B, S, HID), dtype=np.float32)
    for b in range(B):
        out[b] = results[2 * b]["out"] + results[2 * b + 1]["out"] \
            + bvw[None, :]
    return out


def run(inputs, trace=False, **kw):
    nc = get_nc()
    in_maps = make_in_maps(**inputs)
    bkr = run_bass_kernel_spmd(nc, in_maps, list(range(2 * B)), trace=trace, **kw)
    return assemble(bkr.results, np.asarray(inputs["bv"]),
                    np.asarray(inputs["wo"]), np.asarray(inputs["bo"])), bkr


def kernel(**inputs):
    out, _ = run(inputs, trace=False)
    return out


# revision 25
# speedup vs baseline: 1.0968x; 1.0887x over previous
"""MHA kernel for trn2: 8-core SPMD, core c = (batch c//2, head-group c%2 of 8 heads).

Per-core pipeline (all shapes hardcoded for B=4, S=2048, HIDDEN=1024, H=16, DK=DV=64):
  Prologue: K^T per head-pair [128, S] (bf16, bias via ACT Identity per-partition
            bias on PSUM evac), ones-augmented V [sk, 8, 65] (no bias: bv folds
            into bo on the host), Q^T for sq-block 0 only.
  Main loop: per sq-block j (512), per (head hl, sk-tile-pair tt):
            scores^T = K Q^T (2 matmuls), exp on ACT (scale=1/8), mask multiply
            on DVE (bf16, all-SBUF 2x mode), PV accumulation lagging 2 tiles to
            hide the exp->mask latency; the denominator rides the 65th V column.
            Drip-fed per iteration: Q^T projection for block j+1, the previous
            block's normalization (one-hot-selector broadcast matmul + DVE mul)
            and out-projection (pair-packed lhsT, K=128), mask prefetch for j+1.
  Host sums the 2 group partials per batch + bo + bv@wo.
"""

import numpy as np
import ml_dtypes

import concourse.bacc as bacc
import concourse.mybir as mybir
import concourse.tile as tile
from concourse.bass_utils import run_bass_kernel_spmd

B, S, HID, H = 4, 2048, 1024, 16
DK = DV = 64
G = 2              # head groups per batch (8 heads each)
HPC, PAIRS = 8, 4  # heads / head-pairs per core
SQB = 512          # sq block
NJ = S // SQB      # 4
NT = S // 128      # 16 sk tiles
KTN = HID // 128   # 8 hidden k-tiles

F32 = mybir.dt.float32
F32R = mybir.dt.float32r
BF16 = mybir.dt.bfloat16
AF = mybir.ActivationFunctionType


_NC = None


def _build_nc():
    nc = bacc.Bacc("TRN2")
    xq_d = nc.declare_dram_parameter("xqT", [NJ, 128, KTN, SQB], BF16,
                                     isOutput=False)
    xk_d = nc.declare_dram_parameter("xkT", [NJ, 128, KTN, SQB], BF16,
                                     isOutput=False)
    xv_d = nc.declare_dram_parameter("xvT", [NJ, 128, KTN, SQB], BF16,
                                     isOutput=False)
    mk_d = nc.declare_dram_parameter("maskJ", [NJ, 128, NT, SQB], BF16,
                                     isOutput=False)
    wq_d = nc.declare_dram_parameter("wq", [128, KTN, 512], BF16, isOutput=False)
    wk_d = nc.declare_dram_parameter("wk", [128, KTN, 512], BF16, isOutput=False)
    wv_d = nc.declare_dram_parameter("wv", [128, KTN, 512], BF16, isOutput=False)
    bq_d = nc.declare_dram_parameter("bqT", [128, PAIRS], F32, isOutput=False)
    bk_d = nc.declare_dram_parameter("bkT", [128, PAIRS], F32, isOutput=False)
    wo_d = nc.declare_dram_parameter("wo", [PAIRS, 128, HID], BF16, isOutput=False)
    sel_d = nc.declare_dram_parameter("sel", [4, 4 * DV], F32R, isOutput=False)
    out_d = nc.declare_dram_parameter("out", [S, HID], F32, isOutput=True)

    with tile.TileContext(nc) as tc:
        with tc.tile_pool(name="persist", bufs=1) as PP, \
             tc.tile_pool(name="wqp", bufs=1) as WQP, \
             tc.tile_pool(name="mskp", bufs=2) as MP, \
             tc.tile_pool(name="xpool", bufs=2) as XP, \
             tc.tile_pool(name="wpool", bufs=2) as WP, \
             tc.tile_pool(name="ptp", bufs=3) as PTP, \
             tc.tile_pool(name="oup", bufs=2) as OUP, \
             tc.tile_pool(name="onp", bufs=2) as ONP, \
             tc.tile_pool(name="dnp", bufs=2) as DNP, \
             tc.tile_pool(name="dtp", bufs=2) as DTP, \
             tc.tile_pool(name="rcp", bufs=2) as RCP, \
             tc.tile_pool(name="obp", bufs=2) as OBP:
            qT = PP.tile([128, PAIRS, S], BF16, name="qT")
            kT = PP.tile([128, PAIRS, S], BF16, name="kT")
            vA = PP.tile([128, NT, HPC, DV + 1], BF16, name="vA")
            bq_sb = PP.tile([128, PAIRS], F32, name="bq_sb")
            bk_sb = PP.tile([128, PAIRS], F32, name="bk_sb")
            nc.vector.memset(vA[:, :, :, DV:DV + 1], 1.0)
            sel = PP.tile([4, 4 * DV], F32R, name="sel")
            nc.sync.dma_start(bq_sb[:], bq_d[:])
            nc.sync.dma_start(bk_sb[:], bk_d[:])
            wo_sb = PP.tile([128, PAIRS, HID], BF16, name="wo_sb")
            msk0 = MP.tile([128, NT, SQB], BF16, name="msk")
            wq_sb = WQP.tile([128, KTN, 512], BF16, name="wq_sb")

            # ---------------- Prologue: K, V, Q(j=0) projections ----------
            with tc.tile_pool(name="prps", bufs=2, space="PSUM") as PR:
                def qk_proj(xd, w_sb, brow, dstT, n, dma):
                    x_sb = XP.tile([128, KTN, SQB], BF16, name="x_sb")
                    dma(x_sb[:], xd[n])
                    for hp in range(PAIRS):
                        ps = PR.tile([128, SQB], F32, name="ps_qk")
                        for k in range(KTN):
                            nc.tensor.matmul(
                                ps[:], w_sb[:, k, hp * 128:(hp + 1) * 128],
                                x_sb[:, k, :], start=(k == 0), stop=(k == KTN - 1))
                        nc.scalar.activation(
                            dstT[:, hp, n * SQB:(n + 1) * SQB], ps[:],
                            AF.Identity, bias=brow[:, hp:hp + 1])

                wk_sb = WP.tile([128, KTN, 512], BF16, name="w_sb")
                nc.sync.dma_start(wk_sb[:], wk_d[:])
                for n in range(NJ):
                    qk_proj(xk_d, wk_sb, bk_sb, kT, n, nc.sync.dma_start)


                # V: out[sk 128, head, dv] += x.T @ wv (no bias)
                wv_sb = WP.tile([128, KTN, 512], BF16, name="w_sb")
                nc.sync.dma_start(wv_sb[:], wv_d[:])
                for c in range(4):
                    x_sb = XP.tile([128, KTN, SQB], BF16, name="x_sb")
                    nc.sync.dma_start(x_sb[:], xv_d[c])
                    for stl in range(4):
                        st = c * 4 + stl
                        ps = PR.tile([128, HPC, DV], F32, name="ps_v")
                        for k in range(KTN):
                            nc.tensor.matmul(
                                ps[:], x_sb[:, k, stl * 128:(stl + 1) * 128],
                                wv_sb[:, k, :], start=(k == 0),
                                stop=(k == KTN - 1))
                        nc.vector.tensor_copy(vA[:, st, :, 0:DV], ps[:])

                nc.sync.dma_start(wq_sb[:], wq_d[:])
                for n in range(NJ):
                    qk_proj(xq_d, wq_sb, bq_sb, qT, n, nc.sync.dma_start)

                # deferred bulk loads, last on the same ordered sync queue:
                # needed only from the main loop onwards
                nc.sync.dma_start(msk0[:], mk_d[0])
                for hp in range(PAIRS):
                    nc.sync.dma_start(wo_sb[:, hp, :], wo_d[hp])
                nc.sync.dma_start(sel[:], sel_d[:])

            # ---------------- Main loop: attention + drip-fed extras --------
            with tc.tile_pool(name="scps", bufs=2, space="PSUM") as SCP, \
                 tc.tile_pool(name="pvps", bufs=2, space="PSUM") as PVP, \
                 tc.tile_pool(name="opps", bufs=2, space="PSUM") as OPP:

                def bc_step(hl, rcref, oU, oN):
                    def s_bc():
                        hp, r = divmod(hl, 2)
                        pb = 64 * r
                        bc = OPP.tile([128, SQB], F32, name="op")
                        nc.tensor.matmul(bc[0:DV, :],
                                         sel[:, (hl % 4) * DV:
                                             (hl % 4 + 1) * DV],
                                         rcref[hl // 4][:], start=True,
                                         stop=True)
                        nc.vector.tensor_mul(oN[pb:pb + DV, hp, :],
                                             oU[pb:pb + DV, hp, :],
                                             bc[0:DV, :])
                    return s_bc

                def op_step(j, stl, nn, oN):
                    def s_op():
                        st = 4 * j + stl
                        op = OPP.tile([128, SQB], F32, name="op")
                        for hp in range(PAIRS):
                            nc.tensor.matmul(
                                op[:],
                                oN[:, hp, stl * 128:(stl + 1) * 128],
                                wo_sb[:, hp, nn * SQB:(nn + 1) * SQB],
                                start=(hp == 0), stop=(hp == PAIRS - 1))
                        ob = OBP.tile([128, SQB], F32, name="ob")
                        nc.vector.tensor_copy(ob[:], op[:])
                        nc.sync.dma_start(
                            out_d[st * 128:(st + 1) * 128,
                                  nn * SQB:(nn + 1) * SQB], ob[:])
                    return s_op

                def tail_steps(j, rcref, oU, oN, skip_bc=0):
                    steps = [bc_step(hl, rcref, oU, oN)
                             for hl in range(skip_bc, HPC)]
                    steps += [op_step(j, stl, nn, oN)
                              for stl in range(4) for nn in range(2)]
                    return steps

                def emit_recip4(denj, half):
                    rc4 = RCP.tile([4, SQB], F32R, name="rc4")
                    with nc.allow_low_precision(reason="f32r recip"):
                        nc.vector.reciprocal(rc4[:], denj[half][:])
                    return rc4

                def emit_evac(ev):
                    cpv, chl, cpb, chp, cdenj, coU = ev
                    dtmp = DTP.tile([1, SQB], F32, name="dtmp")
                    nc.vector.tensor_copy(dtmp[:], cpv[DV:DV + 1, :])
                    nc.sync.dma_start(
                        cdenj[chl // 4][chl % 4:chl % 4 + 1, :], dtmp[:])
                    nc.vector.tensor_copy(coU[cpb:cpb + DV, chp, :],
                                          cpv[0:DV, :])

                class Carry:
                    __slots__ = ("pv", "pt", "tt", "hl", "pb", "hp",
                                 "denj", "oU", "fin")

                    def __init__(self, pt, tt, hl, pb, hp, denj, oU, fin):
                        self.pv = None
                        self.pt, self.tt, self.hl = pt, tt, hl
                        self.pb, self.hp = pb, hp
                        self.denj, self.oU, self.fin = denj, oU, fin

                def emit_pv(c, pvref):
                    # pvref[hl] caches the PVP tile for this head
                    if c.tt == 0:
                        pvref[c.hl] = PVP.tile([DV + 1, SQB], F32, name="pv")
                    pv = pvref[c.hl]
                    for u in range(2):
                        nc.tensor.matmul(
                            pv[:], vA[:, 2 * c.tt + u, c.hl, :], c.pt[:, u, :],
                            start=(c.tt == 0 and u == 0),
                            stop=(c.fin and u == 1))
                    if c.fin:
                        return (pv, c.hl, c.pb, c.hp, c.denj, c.oU)
                    return None

                carries = []   # FIFO of pending pv emissions (depth 3)
                pvref = {}
                steps = []
                fsteps = []
                pend = None
                rcref = {}     # {0: rc4 heads 0-3, 1: rc4 heads 4-7}

                def drain_one():
                    if len(carries) >= 3:
                        ev = emit_pv(carries.pop(0), pvref)
                        if ev is not None:
                            emit_evac(ev)

                for j in range(NJ):
                    if j == 0:
                        msk = msk0
                    else:
                        msk = msk_next
                    denj = (DNP.tile([4, SQB], F32, name="denja"),
                            DNP.tile([4, SQB], F32, name="denjb"))
                    oU = OUP.tile([128, PAIRS, SQB], BF16, name="oU")
                    oN = ONP.tile([128, PAIRS, SQB], BF16, name="oN")
                    for hl in range(HPC):
                        hp, r = divmod(hl, 2)
                        pb = 64 * r
                        for tt in range(NT // 2):
                            gi = hl * (NT // 2) + tt
                            sc = SCP.tile([128, 2, SQB], F32, name="sc")
                            for u in range(2):
                                t = 2 * tt + u
                                nc.tensor.matmul(
                                    sc[:, u, :],
                                    kT[pb:pb + DK, hp, t * 128:(t + 1) * 128],
                                    qT[pb:pb + DK, hp, j * SQB:(j + 1) * SQB],
                                    start=True, stop=True)
                            drain_one()
                            if tt % 2 == 0:
                                pt2 = PTP.tile([128, 4, SQB], BF16, name="pt")
                            uo = 2 * (tt % 2)
                            nc.scalar.activation(pt2[:, uo:uo + 2, :], sc[:],
                                                 AF.Exp, scale=0.125)
                            if tt % 2 == 1:
                                # one batched mult per tile-pair amortizes the
                                # DVE per-instruction overhead
                                nc.vector.tensor_mul(
                                    pt2[:], pt2[:],
                                    msk[:, 2 * tt - 2:2 * tt + 2, :])
                            carries.append(
                                Carry(pt2[:, uo:uo + 2, :], tt, hl, pb, hp,
                                      denj, oU, tt == NT // 2 - 1))
                            # drip-fed extras, one per iteration slot
                            if gi == 3 and pend is not None:
                                # heads 4-7 recip for the previous block; the
                                # 0-3 half was computed mid-previous-block
                                rcref[1] = emit_recip4(pend[1], 1)
                                steps = tail_steps(pend[0], dict(rcref),
                                                   pend[2], pend[3])
                            elif gi == 36:
                                # heads 0-3 denominators of THIS block are
                                # complete; recip early so the final block can
                                # normalize heads 0-3 in-loop
                                rcref[0] = emit_recip4(denj, 0)
                            elif gi >= 56 and gi % 2 == 0 and j == NJ - 1:
                                if not fsteps:
                                    fsteps = [bc_step(hl, dict(rcref), oU, oN)
                                              for hl in range(4)]
                                fsteps.pop(0)()
                            elif gi == 41 and j < NJ - 1:
                                msk_next = MP.tile([128, NT, SQB], BF16,
                                                   name="msk")
                                nc.gpsimd.dma_start(msk_next[:],
                                                    mk_d[j + 1])
                            elif gi >= 6 and gi % 4 == 2 and steps:
                                steps.pop(0)()
                    while steps:
                        steps.pop(0)()
                    pend = (j, denj, oU, oN)
                # final block's tail, nothing left to overlap with
                while carries:
                    ev = emit_pv(carries.pop(0), pvref)
                    if ev is not None:
                        emit_evac(ev)
                rcref[1] = emit_recip4(pend[1], 1)
                for s in tail_steps(pend[0], dict(rcref), pend[2], pend[3],
                                    skip_bc=4):
                    s()
    nc.finalize()
    return nc


def get_nc():
    global _NC
    if _NC is None:
        _NC = _build_nc()
    return _NC


def make_in_maps(q_hidden_inputs, k_hidden_inputs, v_hidden_inputs, mask,
                 wq, bq, wk, bk, wv, bv, wo, bo):
    f32 = np.float32
    bf16 = ml_dtypes.bfloat16
    in_maps = []
    per_batch = []
    sel = np.zeros((4, 4 * DV), dtype=f32)
    for r in range(4):
        sel[r, r * DV:(r + 1) * DV] = 1.0
    def x_tile(x):
        # [NJ, 128, KTN, SQB] with x3[n, p, k, s] = x[n*SQB+s, k*128+p]
        return np.ascontiguousarray(
            np.asarray(x).reshape(NJ, SQB, KTN, 128).transpose(0, 3, 2, 1)
        ).astype(bf16)

    def w_tile(w_grp):
        # [128, KTN, 512] with w2[p, k, n] = w_grp[k*128+p, n]
        return np.ascontiguousarray(
            w_grp.reshape(KTN, 128, 512).transpose(1, 0, 2)).astype(bf16)

    for b in range(B):
        xqT = x_tile(q_hidden_inputs[b])
        xkT = x_tile(k_hidden_inputs[b])
        xvT = x_tile(v_hidden_inputs[b])
        maskT = mask[b].T.astype(bf16)                        # [sk, sq]
        # maskJ[j, p, t, s] = maskT[t*128+p, j*512+s]
        maskJ = np.ascontiguousarray(
            maskT.reshape(NT, 128, NJ, SQB).transpose(2, 1, 0, 3))
        per_batch.append((xqT, xkT, xvT, maskJ))
    for c in range(2 * B):
        b, g = divmod(c, 2)
        xqT, xkT, xvT, maskJ = per_batch[b]
        hs = slice(g * HPC, (g + 1) * HPC)
        in_maps.append({
            "xqT": xqT, "xkT": xkT, "xvT": xvT, "maskJ": maskJ,
            "wq": w_tile(wq[hs].transpose(1, 0, 2).reshape(HID, 512)),
            "wk": w_tile(wk[hs].transpose(1, 0, 2).reshape(HID, 512)),
            "wv": w_tile(wv[hs].transpose(1, 0, 2).reshape(HID, 512)),
            # bqT[p, hp] = bq[g*8 + 2*hp + p//64, p%64]
            "bqT": np.ascontiguousarray(
                bq[hs].reshape(PAIRS, 128).T, dtype=f32),
            "bkT": np.ascontiguousarray(
                bk[hs].reshape(PAIRS, 128).T, dtype=f32),
            "wo": np.ascontiguousarray(
                wo[g * 512:(g + 1) * 512, :].reshape(PAIRS, 128, HID)
            ).astype(bf16),
            "sel": sel,
        })
    return in_maps


def assemble(results, bv, wo, bo):
    # v-bias contribution folds through softmax: out_h = rawPV_h/denom + bv_h,
    # so sum_h bv_h @ wo_h is a constant row added once per batch.
    bvw = (bv.astype(np.float32).reshape(-1) @ wo.astype(np.float32)
           + bo.astype(np.float32))
    out = np.empty((B, S, HID), dtype=np.float32)
    for b in range(B):
        out[b] = results[2 * b]["out"] + results[2 * b + 1]["out"] \
            + bvw[None, :]
    return out


def run(inputs, trace=False, **kw):
    nc = get_nc()
    in_maps = make_in_maps(**inputs)
    bkr = run_bass_kernel_spmd(nc, in_maps, list(range(2 * B)), trace=trace, **kw)
    return assemble(bkr.results, np.asarray(inputs["bv"]),
                    np.asarray(inputs["wo"]), np.asarray(inputs["bo"])), bkr


def kernel(**inputs):
    out, _ = run(inputs, trace=False)
    return out


# revision 28
# speedup vs baseline: 1.1706x; 1.0673x over previous
"""MHA kernel for trn2: 8-core SPMD, core c = (batch c//2, head-group c%2 of 8 heads).

Per-core pipeline (all shapes hardcoded for B=4, S=2048, HIDDEN=1024, H=16, DK=DV=64):
  Prologue: K^T per head-pair [128, S] (bf16, bias via ACT Identity per-partition
            bias on PSUM evac), ones-augmented V [sk, 8, 65] (no bias: bv folds
            into bo on the host), Q^T for sq-block 0 only.
  Main loop: per sq-block j (512), per (head hl, sk-tile-pair tt):
            scores^T = K Q^T (2 matmuls), exp on ACT (scale=1/8), mask multiply
            on DVE (bf16, all-SBUF 2x mode), PV accumulation lagging 2 tiles to
            hide the exp->mask latency; the denominator rides the 65th V column.
            Drip-fed per iteration: Q^T projection for block j+1, the previous
            block's normalization (one-hot-selector broadcast matmul + DVE mul)
            and out-projection (pair-packed lhsT, K=128), mask prefetch for j+1.
  Host sums the 2 group partials per batch + bo + bv@wo.
"""

import numpy as np
import ml_dtypes

import concourse.bacc as bacc
import concourse.mybir as mybir
import concourse.tile as tile
from concourse.bass_utils import run_bass_kernel_spmd

B, S, HID, H = 4, 2048, 1024, 16
DK = DV = 64
G = 2              # head groups per batch (8 heads each)
HPC, PAIRS = 8, 4  # heads / head-pairs per core
SQB = 512          # sq block
NJ = S // SQB      # 4
NT = S // 128      # 16 sk tiles
KTN = HID // 128   # 8 hidden k-tiles

F32 = mybir.dt.float32
F32R = mybir.dt.float32r
BF16 = mybir.dt.bfloat16
AF = mybir.ActivationFunctionType


_NC = None


def _build_nc():
    nc = bacc.Bacc("TRN2")
    xq_d = nc.declare_dram_parameter("xqT", [NJ, 128, KTN, SQB], BF16,
                                     isOutput=False)
    xk_d = nc.declare_dram_parameter("xkT", [NJ, 128, KTN, SQB], BF16,
                                     isOutput=False)
    xv_d = nc.declare_dram_parameter("xvT", [NJ, 128, KTN, SQB], BF16,
                                     isOutput=False)
    mk_d = nc.declare_dram_parameter("maskJ", [NJ, 128, NT, SQB], BF16,
                                     isOutput=False)
    wq_d = nc.declare_dram_parameter("wq", [128, KTN, 512], BF16, isOutput=False)
    wk_d = nc.declare_dram_parameter("wk", [128, KTN, 512], BF16, isOutput=False)
    wv_d = nc.declare_dram_parameter("wv", [128, KTN, 512], BF16, isOutput=False)
    bq_d = nc.declare_dram_parameter("bqT", [128, PAIRS], F32, isOutput=False)
    bk_d = nc.declare_dram_parameter("bkT", [128, PAIRS], F32, isOutput=False)
    wo_d = nc.declare_dram_parameter("wo", [PAIRS, 128, HID], BF16, isOutput=False)
    sel_d = nc.declare_dram_parameter("sel", [4, 4 * DV], F32R, isOutput=False)
    out_d = nc.declare_dram_parameter("out", [S, HID], F32, isOutput=True)

    with tile.TileContext(nc) as tc:
        with tc.tile_pool(name="persist", bufs=1) as PP, \
             tc.tile_pool(name="wqp", bufs=1) as WQP, \
             tc.tile_pool(name="mskp", bufs=2) as MP, \
             tc.tile_pool(name="xpool", bufs=2) as XP, \
             tc.tile_pool(name="wpool", bufs=2) as WP, \
             tc.tile_pool(name="ptp", bufs=3) as PTP, \
             tc.tile_pool(name="oup", bufs=2) as OUP, \
             tc.tile_pool(name="onp", bufs=2) as ONP, \
             tc.tile_pool(name="dnp", bufs=2) as DNP, \
             tc.tile_pool(name="dtp", bufs=2) as DTP, \
             tc.tile_pool(name="rcp", bufs=2) as RCP, \
             tc.tile_pool(name="obp", bufs=2) as OBP:
            qT = PP.tile([128, PAIRS, S], BF16, name="qT")
            kT = PP.tile([128, PAIRS, S], BF16, name="kT")
            vA = PP.tile([128, NT, HPC, DV + 1], BF16, name="vA")
            bq_sb = PP.tile([128, PAIRS], F32, name="bq_sb")
            bk_sb = PP.tile([128, PAIRS], F32, name="bk_sb")
            nc.vector.memset(vA[:, :, :, DV:DV + 1], 1.0)
            sel = PP.tile([4, 4 * DV], F32R, name="sel")
            nc.sync.dma_start(bq_sb[:], bq_d[:])
            nc.sync.dma_start(bk_sb[:], bk_d[:])
            wo_sb = PP.tile([128, PAIRS, HID], BF16, name="wo_sb")
            msk0 = MP.tile([128, NT, SQB], BF16, name="msk")
            wq_sb = WQP.tile([128, KTN, 512], BF16, name="wq_sb")

            # ---------------- Prologue: K, V, Q(j=0) projections ----------
            with tc.tile_pool(name="prps", bufs=2, space="PSUM") as PR:
                def qk_proj(xd, w_sb, brow, dstT, n, dma):
                    x_sb = XP.tile([128, KTN, SQB], BF16, name="x_sb")
                    dma(x_sb[:], xd[n])
                    for hp in range(PAIRS):
                        ps = PR.tile([128, SQB], F32, name="ps_qk")
                        for k in range(KTN):
                            nc.tensor.matmul(
                                ps[:], w_sb[:, k, hp * 128:(hp + 1) * 128],
                                x_sb[:, k, :], start=(k == 0), stop=(k == KTN - 1))
                        nc.scalar.activation(
                            dstT[:, hp, n * SQB:(n + 1) * SQB], ps[:],
                            AF.Identity, bias=brow[:, hp:hp + 1])

                wk_sb = WP.tile([128, KTN, 512], BF16, name="w_sb")
                nc.sync.dma_start(wk_sb[:], wk_d[:])
                for n in range(NJ):
                    qk_proj(xk_d, wk_sb, bk_sb, kT, n, nc.sync.dma_start)


                # V: out[sk 128, head, dv] += x.T @ wv (no bias)
                wv_sb = WP.tile([128, KTN, 512], BF16, name="w_sb")
                nc.sync.dma_start(wv_sb[:], wv_d[:])
                for c in range(4):
                    x_sb = XP.tile([128, KTN, SQB], BF16, name="x_sb")
                    nc.sync.dma_start(x_sb[:], xv_d[c])
                    for stl in range(4):
                        st = c * 4 + stl
                        ps = PR.tile([128, HPC, DV], F32, name="ps_v")
                        for k in range(KTN):
                            nc.tensor.matmul(
                                ps[:], x_sb[:, k, stl * 128:(stl + 1) * 128],
                                wv_sb[:, k, :], start=(k == 0),
                                stop=(k == KTN - 1))
                        nc.vector.tensor_copy(vA[:, st, :, 0:DV], ps[:])

                nc.sync.dma_start(wq_sb[:], wq_d[:])
                for n in range(NJ):
                    qk_proj(xq_d, wq_sb, bq_sb, qT, n, nc.sync.dma_start)

                # deferred bulk loads, last on the same ordered sync queue:
                # needed only from the main loop onwards
                nc.sync.dma_start(msk0[:], mk_d[0])
                for hp in range(PAIRS):
                    nc.sync.dma_start(wo_sb[:, hp, :], wo_d[hp])
                nc.sync.dma_start(sel[:], sel_d[:])

            # ---------------- Main loop: attention + drip-fed extras --------
            with tc.tile_pool(name="scps", bufs=2, space="PSUM") as SCP, \
                 tc.tile_pool(name="pvps", bufs=2, space="PSUM") as PVP, \
                 tc.tile_pool(name="opps", bufs=2, space="PSUM") as OPP:

                def bc_step(hl, rcref, oU, oN):
                    def s_bc():
                        hp, r = divmod(hl, 2)
                        pb = 64 * r
                        bc = OPP.tile([128, SQB], F32, name="op")
                        nc.tensor.matmul(bc[0:DV, :],
                                         sel[:, (hl % 4) * DV:
                                             (hl % 4 + 1) * DV],
                                         rcref[hl // 4][:], start=True,
                                         stop=True)
                        nc.vector.tensor_mul(oN[pb:pb + DV, hp, :],
                                             oU[pb:pb + DV, hp, :],
                                             bc[0:DV, :])
                    return s_bc

                def op_step(j, stl, nn, oN):
                    def s_op():
                        st = 4 * j + stl
                        op = OPP.tile([128, SQB], F32, name="op")
                        for hp in range(PAIRS):
                            nc.tensor.matmul(
                                op[:],
                                oN[:, hp, stl * 128:(stl + 1) * 128],
                                wo_sb[:, hp, nn * SQB:(nn + 1) * SQB],
                                start=(hp == 0), stop=(hp == PAIRS - 1))
                        ob = OBP.tile([128, SQB], F32, name="ob")
                        nc.vector.tensor_copy(ob[:], op[:])
                        nc.sync.dma_start(
                            out_d[st * 128:(st + 1) * 128,
                                  nn * SQB:(nn + 1) * SQB], ob[:])
                    return s_op

                def tail_steps(j, rcref, oU, oN, skip_bc=0):
                    steps = [bc_step(hl, rcref, oU, oN)
                             for hl in range(skip_bc, HPC)]
                    steps += [op_step(j, stl, nn, oN)
                              for stl in range(4) for nn in range(2)]
                    return steps

                def emit_recip4(denj, half):
                    rcf = DTP.tile([4, SQB], F32, name="rcf")
                    rc4 = RCP.tile([4, SQB], F32R, name="rc4")
                    with nc.allow_low_precision(reason="f32r recip"):
                        nc.vector.reciprocal_approx_fast(rcf[:],
                                                         denj[half][:])
                        nc.vector.tensor_copy(rc4[:], rcf[:])
                    return rc4

                def emit_evac(ev):
                    cpv, chl, cpb, chp, cdenj, coU = ev
                    dtmp = DTP.tile([1, SQB], F32, name="dtmp")
                    nc.vector.tensor_copy(dtmp[:], cpv[DV:DV + 1, :])
                    nc.sync.dma_start(
                        cdenj[chl // 4][chl % 4:chl % 4 + 1, :], dtmp[:])
                    nc.vector.tensor_copy(coU[cpb:cpb + DV, chp, :],
                                          cpv[0:DV, :])

                class Carry:
                    __slots__ = ("pv", "pt", "tt", "hl", "pb", "hp",
                                 "denj", "oU", "fin")

                    def __init__(self, pt, tt, hl, pb, hp, denj, oU, fin):
                        self.pv = None
                        self.pt, self.tt, self.hl = pt, tt, hl
                        self.pb, self.hp = pb, hp
                        self.denj, self.oU, self.fin = denj, oU, fin

                def emit_pv(c, pvref):
                    # pvref[hl] caches the PVP tile for this head
                    if c.tt == 0:
                        pvref[c.hl] = PVP.tile([DV + 1, SQB], F32, name="pv")
                    pv = pvref[c.hl]
                    for u in range(2):
                        nc.tensor.matmul(
                            pv[:], vA[:, 2 * c.tt + u, c.hl, :], c.pt[:, u, :],
                            start=(c.tt == 0 and u == 0),
                            stop=(c.fin and u == 1))
                    if c.fin:
                        return (pv, c.hl, c.pb, c.hp, c.denj, c.oU)
                    return None

                carries = []   # FIFO of pending pv emissions (depth 3)
                pvref = {}
                steps = []
                fsteps = []
                pend = None
                rcref = {}     # {0: rc4 heads 0-3, 1: rc4 heads 4-7}

                def drain_one():
                    if len(carries) >= 3:
                        ev = emit_pv(carries.pop(0), pvref)
                        if ev is not None:
                            emit_evac(ev)

                for j in range(NJ):
                    if j == 0:
                        msk = msk0
                    else:
                        msk = msk_next
                    denj = (DNP.tile([4, SQB], F32, name="denja"),
                            DNP.tile([4, SQB], F32, name="denjb"))
                    oU = OUP.tile([128, PAIRS, SQB], BF16, name="oU")
                    oN = ONP.tile([128, PAIRS, SQB], BF16, name="oN")
                    for hl in range(HPC):
                        hp, r = divmod(hl, 2)
                        pb = 64 * r
                        for tt in range(NT // 2):
                            gi = hl * (NT // 2) + tt
                            sc = SCP.tile([128, 2, SQB], F32, name="sc")
                            for u in range(2):
                                t = 2 * tt + u
                                nc.tensor.matmul(
                                    sc[:, u, :],
                                    kT[pb:pb + DK, hp, t * 128:(t + 1) * 128],
                                    qT[pb:pb + DK, hp, j * SQB:(j + 1) * SQB],
                                    start=True, stop=True)
                            drain_one()
                            if tt % 2 == 0:
                                pt2 = PTP.tile([128, 4, SQB], BF16, name="pt")
                            uo = 2 * (tt % 2)
                            nc.scalar.activation(pt2[:, uo:uo + 2, :], sc[:],
                                                 AF.Exp, scale=0.125)
                            if tt % 2 == 1:
                                # one batched mult per tile-pair amortizes the
                                # DVE per-instruction overhead
                                nc.vector.tensor_mul(
                                    pt2[:], pt2[:],
                                    msk[:, 2 * tt - 2:2 * tt + 2, :])
                            carries.append(
                                Carry(pt2[:, uo:uo + 2, :], tt, hl, pb, hp,
                                      denj, oU, tt == NT // 2 - 1))
                            # drip-fed extras, one per iteration slot
                            if gi == 3 and pend is not None:
                                # heads 4-7 recip for the previous block; the
                                # 0-3 half was computed mid-previous-block
                                rcref[1] = emit_recip4(pend[1], 1)
                                steps = tail_steps(pend[0], dict(rcref),
                                                   pend[2], pend[3])
                            elif gi == 36:
                                # heads 0-3 denominators of THIS block are
                                # complete; recip early so the final block can
                                # normalize heads 0-3 in-loop
                                rcref[0] = emit_recip4(denj, 0)
                            elif gi >= 56 and gi % 2 == 0 and j == NJ - 1:
                                if not fsteps:
                                    fsteps = [bc_step(hl, dict(rcref), oU, oN)
                                              for hl in range(4)]
                                fsteps.pop(0)()
                            elif gi == 41 and j < NJ - 1:
                                msk_next = MP.tile([128, NT, SQB], BF16,
                                                   name="msk")
                                nc.gpsimd.dma_start(msk_next[:],
                                                    mk_d[j + 1])
                            elif gi >= 6 and gi % 4 == 2 and steps:
                                steps.pop(0)()
                    while steps:
                        steps.pop(0)()
                    pend = (j, denj, oU, oN)
                # final block's tail, nothing left to overlap with
                while carries:
                    ev = emit_pv(carries.pop(0), pvref)
                    if ev is not None:
                        emit_evac(ev)
                rcref[1] = emit_recip4(pend[1], 1)
                for s in tail_steps(pend[0], dict(rcref), pend[2], pend[3],
                                    skip_bc=4):
                    s()
    nc.finalize()
    return nc


def get_nc():
    global _NC
    if _NC is None:
        _NC = _build_nc()
    return _NC


def make_in_maps(q_hidden_inputs, k_hidden_inputs, v_hidden_inputs, mask,
                 wq, bq, wk, bk, wv, bv, wo, bo):
    f32 = np.float32
    bf16 = ml_dtypes.bfloat16
    in_maps = []
    per_batch = []
    sel = np.zeros((4, 4 * DV), dtype=f32)
    for r in range(4):
        sel[r, r * DV:(r + 1) * DV] = 1.0
    def x_tile(x):
        # [NJ, 128, KTN, SQB] with x3[n, p, k, s] = x[n*SQB+s, k*128+p]
        return np.ascontiguousarray(
            np.asarray(x).reshape(NJ, SQB, KTN, 128).transpose(0, 3, 2, 1)
        ).astype(bf16)

    def w_tile(w_grp):
        # [128, KTN, 512] with w2[p, k, n] = w_grp[k*128+p, n]
        return np.ascontiguousarray(
            w_grp.reshape(KTN, 128, 512).transpose(1, 0, 2)).astype(bf16)

    for b in range(B):
        xqT = x_tile(q_hidden_inputs[b])
        xkT = x_tile(k_hidden_inputs[b])
        xvT = x_tile(v_hidden_inputs[b])
        maskT = mask[b].T.astype(bf16)                        # [sk, sq]
        # maskJ[j, p, t, s] = maskT[t*128+p, j*512+s]
        maskJ = np.ascontiguousarray(
            maskT.reshape(NT, 128, NJ, SQB).transpose(2, 1, 0, 3))
        per_batch.append((xqT, xkT, xvT, maskJ))
    for c in range(2 * B):
        b, g = divmod(c, 2)
        xqT, xkT, xvT, maskJ = per_batch[b]
        hs = slice(g * HPC, (g + 1) * HPC)
        in_maps.append({
            "xqT": xqT, "xkT": xkT, "xvT": xvT, "maskJ": maskJ,
            "wq": w_tile(wq[hs].transpose(1, 0, 2).reshape(HID, 512)),
            "wk": w_tile(wk[hs].transpose(1, 0, 2).reshape(HID, 512)),
            "wv": w_tile(wv[hs].transpose(1, 0, 2).reshape(HID, 512)),
            # bqT[p, hp] = bq[g*8 + 2*hp + p//64, p%64]
            "bqT": np.ascontiguousarray(
                bq[hs].reshape(PAIRS, 128).T, dtype=f32),
            "bkT": np.ascontiguousarray(
                bk[hs].reshape(PAIRS, 128).T, dtype=f32),
            "wo": np.ascontiguousarray(
                wo[g * 512:(g + 1) * 512, :].reshape(PAIRS, 128, HID)
            ).astype(bf16),
            "sel": sel,
        })
    return in_maps


def assemble(results, bv, wo, bo):
    # v-bias contribution folds through softmax: out_h = rawPV_h/denom + bv_h,
    # so sum_h bv_h @ wo_h is a constant row added once per batch.
    bvw = (bv.astype(np.float32).reshape(-1) @ wo.astype(np.float32)
           + bo.astype(np.float32))
    out = np.empty((B, S, HID), dtype=np.float32)
    for b in range(B):
        out[b] = results[2 * b]["out"] + results[2 * b + 1]["out"] \
            + bvw[None, :]
    return out


def run(inputs, trace=False, **kw):
    nc = get_nc()
    in_maps = make_in_maps(**inputs)
    bkr = run_bass_kernel_spmd(nc, in_maps, list(range(2 * B)), trace=trace, **kw)
    return assemble(bkr.results, np.asarray(inputs["bv"]),
                    np.asarray(inputs["wo"]), np.asarray(inputs["bo"])), bkr


def kernel(**inputs):
    out, _ = run(inputs, trace=False)
    return out
